# revision 1
# baseline (speedup 1.0000x reference)
"""Multi-headed attention TRN2 Bass kernel.

Problem: B=2, S=2048, D=1024, H=16 heads (dh=64), fp32, bool mask.

Sharding (8 cores): data-parallel over B (2) x tensor-parallel over heads
(4 heads / 256 features per core). Each core computes its head-group's
q/k/v projections, masked softmax attention, and a partial output
projection (Wo columns for its heads). Host sums the 4 partials per batch
element (the TP all-reduce) and adds the bias.

Per-core kernel design (all layouts transposed, i.e. feature-major):
  phase 1: qT/kT pair tiles [128 feat, S] and v tiles [128 s, 256 feat]
           via fp32r matmuls from xT [D, S].
  phase 2: scores_T[k, q] per head-pair via row-packed K=64 fp32r matmuls
           (head a on partitions 0:64, head b on 64:128, concurrent on PE).
  phase 3: exp on ACT (PSUM->fp16 SBUF), multiplicative mask on DVE
           (keep mask, fp16, broadcast over the 2 packed heads), then
           ctx_T accumulation (fp16 matmuls, col-packed pair) plus
           denominators via all-ones matmuls. Softmax normalization by
           reciprocal-multiply at PSUM eviction.
  phase 4: partial outT [D, S] = WoT.T @ ctx_T via fp32r matmuls.

No max-subtraction in softmax: scores are ~N(0,1) (|s| < ~7), exp is
computed in fp32->fp16 which is exact enough (validated 4e-4 rel err
end to end vs the fp32 reference).
"""

import math
from contextlib import ExitStack

import numpy as np

import concourse.mybir as mybir
import concourse.tile as tile
from concourse import bacc
from concourse.bass_utils import run_bass_kernel_spmd

B, S, D, H = 2, 2048, 1024, 16
DH = D // H                 # 64
NCORES = 8
GROUPS = NCORES // B        # 4 head-groups per batch element
FPC = D // GROUPS           # 256 features (4 heads) per core
P = 128
SC = 512                    # q/s chunk (free dim of most matmuls)
NQC = S // SC               # 4
NKT = S // P                # 16 k-position tiles
NDT = D // P                # 8 contraction tiles over D

F32 = mybir.dt.float32
F32R = mybir.dt.float32r
F16 = mybir.dt.float16

EXP = mybir.ActivationFunctionType.Exp
MULT = mybir.AluOpType.mult


def _r(ap):
    return ap.bitcast(F32R)


# dev bisection knob: "full", "dma", "noattn", "noctx", "nomask"
VARIANT = "full"
CTX_BUFS = 2  # double-buffer ctx/denom PSUM banks


def _emit(ctx: ExitStack, tc: tile.TileContext, xT, wqT, wkT, wvT, woT, keepT, outT):
    nc = tc.nc

    const = ctx.enter_context(tc.tile_pool(name="const", bufs=1))
    sb = ctx.enter_context(tc.tile_pool(name="sb", bufs=1))
    xtp = ctx.enter_context(tc.tile_pool(name="xtp", bufs=2))
    keepp = ctx.enter_context(tc.tile_pool(name="keepp", bufs=2))
    wp = ctx.enter_context(tc.tile_pool(name="wp", bufs=3))
    stg = ctx.enter_context(tc.tile_pool(name="stg", bufs=3))
    ps = ctx.enter_context(tc.tile_pool(name="ps", bufs=1, space="PSUM"))

    # ---- constants / weights in SBUF ----
    wq_sb = const.tile([P, NDT, FPC], F32R)
    nc.sync.dma_start(wq_sb[:], wqT[:])
    wk_sb = const.tile([P, NDT, FPC], F32R)
    nc.sync.dma_start(wk_sb[:], wkT[:])
    wv_sb = const.tile([P, NDT, FPC], F32R)
    nc.sync.dma_start(wv_sb[:], wvT[:])
    wo_sb = const.tile([P, FPC // P, D], F32R)
    nc.sync.dma_start(wo_sb[:], woT[:])
    ones_bc = const.tile([P, DH], F32)
    nc.vector.memset(ones_bc[:], 1.0 / DH)

    # ---- persistent activations ----
    q_sb = [sb.tile([P, S], F32R, name=f"q_sb{i}") for i in range(2)]
    k_sb = [sb.tile([P, S], F32R, name=f"k_sb{i}") for i in range(2)]
    v_sb = [sb.tile([P, 2, 192], F16, name=f"v_sb{i}") for i in range(NKT)]
    ctx_sb = [sb.tile([P, S], F32R, name=f"ctx_sb{i}") for i in range(2)]

    # ---- phase 1: projections ----
    for sc in range(NQC):
        xt_sc = xtp.tile([P, NDT, SC], F32R, tag="xt", name=f"xt_{sc}")
        nc.sync.dma_start(xt_sc[:], xT[sc])
        if VARIANT == "dma":
            continue
        for pair in range(2):
            for wi, (w_sb, dst) in enumerate(((wq_sb, q_sb), (wk_sb, k_sb))):
                mm = ps.tile([P, SC], F32, tag=("ctxX", "ctxY")[(2 * pair + wi) % 2],
                             bufs=CTX_BUFS, name=f"qk_{sc}_{pair}_{wi}")
                for dt in range(NDT):
                    nc.tensor.matmul(
                        mm[:],
                        w_sb[:, dt, pair * P:(pair + 1) * P],
                        xt_sc[:, dt, :],
                        start=(dt == 0),
                        stop=(dt == NDT - 1),
                    )
                nc.vector.tensor_copy(dst[pair][:, sc * SC:(sc + 1) * SC], mm[:])
        for ssub in range(SC // P):
            kt = sc * (SC // P) + ssub
            vm = ps.tile([P, FPC], F32, tag=("ctxX", "ctxY")[kt % 2], bufs=CTX_BUFS,
                         name=f"v_{kt}")
            for dt in range(NDT):
                nc.tensor.matmul(
                    vm[:],
                    xt_sc[:, dt, ssub * P:(ssub + 1) * P],
                    wv_sb[:, dt, :],
                    start=(dt == 0),
                    stop=(dt == NDT - 1),
                )
            for pr in range(2):
                nc.vector.tensor_copy(v_sb[kt][:, pr, 0:DH],
                                      vm[:, pr * P:pr * P + DH])
                nc.vector.tensor_copy(v_sb[kt][:, pr, 2 * DH:3 * DH],
                                      vm[:, pr * P + DH:(pr + 1) * P])
            nc.vector.memset(v_sb[kt][:, :, DH:2 * DH], 1.0)

    # ---- phases 2+3: attention ----
    for qc in range(NQC):
        keep_sb = keepp.tile([P, NKT, SC], F16, tag="keep", name=f"keep_{qc}")
        nc.scalar.dma_start(keep_sb[:], keepT[qc])
        if VARIANT in ("dma", "noattn"):
            continue
        for pair in range(2):
            # bank Y: ctx_a on [0:64], denom_b on [64:128]
            # bank X: denom_a on [0:64], ctx_b on [64:128]
            # Two independent accumulation regions share each bank, so no
            # start/stop groups: memset to zero, then accumulate without
            # start (first PE write either overwrites (has_written clear)
            # or adds to zero (has_written stale-set) - correct either way).
            ctx_y = ps.tile([P, SC], F32, tag="ctxY", bufs=CTX_BUFS, name=f"ctxY_{qc}_{pair}")
            ctx_x = ps.tile([P, SC], F32, tag="ctxX", bufs=CTX_BUFS, name=f"ctxX_{qc}_{pair}")

            for kt in range(NKT):
                sc_a = ps.tile([P, SC], F32, tag="scA", bufs=2, name=f"sa_{qc}_{pair}_{kt}")
                sc_b = ps.tile([P, SC], F32, tag="scB", bufs=2, name=f"sb_{qc}_{pair}_{kt}")
                ksl = slice(kt * P, (kt + 1) * P)
                qsl = slice(qc * SC, (qc + 1) * SC)
                nc.tensor.matmul(
                    sc_a[:],
                    k_sb[pair][0:DH, ksl],
                    q_sb[pair][0:DH, qsl],
                    start=True, stop=True,
                )
                nc.tensor.matmul(
                    sc_b[:],
                    k_sb[pair][DH:P, ksl],
                    q_sb[pair][DH:P, qsl],
                    start=True, stop=True,
                    tile_position=(64, 0),
                )
                w = wp.tile([P, 2 * SC], F16, tag="w", name=f"w_{qc}_{pair}_{kt}")
                nc.scalar.activation(w[:, 0:SC], sc_a[:], EXP)
                nc.scalar.activation(w[:, SC:2 * SC], sc_b[:], EXP)
                if VARIANT != "nomask":
                    w3 = w[:].rearrange("p (h q) -> p h q", h=2)
                    kb = keep_sb[:, kt, :][:, None, :].to_broadcast((P, 2, SC))
                    eng = nc.gpsimd if kt % 4 == 3 else nc.vector
                    eng.tensor_tensor(w3, w3, kb, MULT)
                if VARIANT == "noctx":
                    continue
                vt = v_sb[kt]
                first, last = kt == 0, kt == NKT - 1
                nc.tensor.matmul(
                    ctx_y[:], vt[:, pair, 0:2 * DH], w[:, 0:SC],
                    start=first, stop=last,
                )
                nc.tensor.matmul(
                    ctx_x[:], vt[:, pair, DH:3 * DH], w[:, SC:2 * SC],
                    start=first, stop=last,
                )
            recip = stg.tile([P, SC], F32, tag="recip", name=f"recip_{qc}_{pair}")
            nc.vector.reciprocal(recip[0:DH, :], ctx_x[0:DH, :])
            nc.vector.reciprocal(recip[DH:P, :], ctx_y[DH:P, :])
            bc_a = ps.tile([P, SC], F32, tag="scA", bufs=2, name=f"bca_{qc}_{pair}")
            nc.tensor.matmul(
                bc_a[0:DH, :], ones_bc[DH:P, 0:DH], recip[DH:P, :],
                start=True, stop=True, tile_position=(64, 0),
            )
            bc_b = ps.tile([P, SC], F32, tag="scB", bufs=2, name=f"bcb_{qc}_{pair}")
            nc.tensor.matmul(
                bc_b[DH:P, :], ones_bc[0:DH, 0:DH], recip[0:DH, :],
                start=True, stop=True, tile_position=(0, 64),
            )
            rcp2 = stg.tile([P, SC], F32, tag="recip2", name=f"rcp2_{qc}_{pair}")
            nc.vector.tensor_copy(rcp2[0:DH, :], bc_a[0:DH, :])
            nc.vector.tensor_copy(rcp2[DH:P, :], bc_b[DH:P, :])
            qsl = slice(qc * SC, (qc + 1) * SC)
            nc.vector.tensor_tensor(
                ctx_sb[pair][0:DH, qsl], ctx_y[0:DH, :], rcp2[0:DH, :], MULT)
            nc.vector.tensor_tensor(
                ctx_sb[pair][DH:P, qsl], ctx_x[DH:P, :], rcp2[DH:P, :], MULT)

    # ---- phase 4: output projection (partial) ----
    for ft in range(D // P):
        st = stg.tile([P, NQC, SC], F32, tag="stage", bufs=2, name=f"st_{ft}")
        for sc in range(NQC):
            om = ps.tile([P, SC], F32, tag=("ctxX", "ctxY")[sc % 2], bufs=CTX_BUFS, name=f"o_{ft}_{sc}")
            if VARIANT in ("dma",):
                nc.vector.memset(om[:], 0.0)
            else:
              for ph in range(FPC // P):
                nc.tensor.matmul(
                    om[:],
                    wo_sb[:, ph, ft * P:(ft + 1) * P],
                    ctx_sb[ph][:, sc * SC:(sc + 1) * SC],
                    start=(ph == 0),
                    stop=(ph == FPC // P - 1),
                )  # noqa
            nc.vector.tensor_copy(st[:, sc, :], om[:])
        nc.scalar.dma_start(outT[ft], st[:])


def build():
    nc = bacc.Bacc("TRN2", target_bir_lowering=False, debug=False, num_devices=NCORES)
    # all inputs pre-tiled on the host so every DMA line is contiguous
    xT = nc.dram_tensor("xT", [NQC, P, NDT, SC], F32R, kind="ExternalInput").ap()
    wqT = nc.dram_tensor("wqT", [P, NDT, FPC], F32R, kind="ExternalInput").ap()
    wkT = nc.dram_tensor("wkT", [P, NDT, FPC], F32R, kind="ExternalInput").ap()
    wvT = nc.dram_tensor("wvT", [P, NDT, FPC], F32R, kind="ExternalInput").ap()
    woT = nc.dram_tensor("woT", [P, FPC // P, D], F32R, kind="ExternalInput").ap()
    keepT = nc.dram_tensor("keepT", [NQC, P, NKT, SC], F16, kind="ExternalInput").ap()
    outT = nc.dram_tensor("outT", [D // P, P, NQC, SC], F32, kind="ExternalOutput").ap()
    with tile.TileContext(nc) as tc, ExitStack() as ctx:
        _emit(ctx, tc, xT, wqT, wkT, wvT, woT, keepT, outT)
    nc.compile()
    return nc


def make_in_maps(query, mask, Wq, Wk, Wv, Wo):
    scale = 1.0 / math.sqrt(DH)
    in_maps = []
    for b in range(B):
        # xT tiled: [NQC, P, NDT, SC]; element (sc, p, dt, s) = x[sc*SC+s, dt*P+p]
        xt = query[b].astype(np.float32).T.reshape(NDT, P, NQC, SC)
        xT = np.ascontiguousarray(xt.transpose(2, 1, 0, 3))
        # keep tiled: [NQC, P, NKT, SC]; element (qc, p, kt, q) = keep[kt*P+p, qc*SC+q]
        kp = (~mask[b]).T.astype(np.float16).reshape(NKT, P, NQC, SC)
        keepT = np.ascontiguousarray(kp.transpose(2, 1, 0, 3))
        for g in range(GROUPS):
            f0 = g * FPC
            def pack_w(wT):  # [D, FPC] -> [P, NDT, FPC]
                return np.ascontiguousarray(
                    wT.reshape(NDT, P, FPC).transpose(1, 0, 2))
            in_maps.append({
                "xT": xT,
                "wqT": pack_w((Wq[f0:f0 + FPC, :] * scale).T.astype(np.float32)),
                "wkT": pack_w(Wk[f0:f0 + FPC, :].T.astype(np.float32)),
                "wvT": pack_w(Wv[f0:f0 + FPC, :].T.astype(np.float32)),
                "woT": np.ascontiguousarray(
                    Wo[:, f0:f0 + FPC].T.astype(np.float32)
                    .reshape(FPC // P, P, D).transpose(1, 0, 2)),
                "keepT": keepT,
            })
    return in_maps


_NC_CACHE = {}


def _get_nc():
    if "nc" not in _NC_CACHE:
        _NC_CACHE["nc"] = build()
    return _NC_CACHE["nc"]


def gather(results, bo):
    out = np.empty((B, S, D), dtype=np.float32)
    for b in range(B):
        acc = results[b * GROUPS]["outT"].astype(np.float32).copy()
        for g in range(1, GROUPS):
            acc += results[b * GROUPS + g]["outT"]
        out[b] = acc.reshape(D, S).T + bo.astype(np.float32)
    return out


def kernel(query, mask, Wq, Wk, Wv, Wo, bo, **kwargs):
    nc = _get_nc()
    in_maps = make_in_maps(np.asarray(query), np.asarray(mask), np.asarray(Wq),
                           np.asarray(Wk), np.asarray(Wv), np.asarray(Wo))
    res = run_bass_kernel_spmd(nc, in_maps, list(range(NCORES)))
    return gather(res.results, np.asarray(bo))



# revision 3
# speedup vs baseline: 2.2221x; 2.2221x over previous
"""Multi-headed attention TRN2 Bass kernel (v2).

Problem: B=2, S=2048, D=1024, H=16 heads (dh=64), fp32 in/out, bool mask.

Sharding (8 cores): data-parallel over B (2) x tensor-parallel over heads
(4 heads / 256 features per core). Each core computes its head-group's
q/k/v projections, masked softmax attention, and a partial output
projection (Wo columns for its heads). Host sums the 4 partials per batch
element (the TP all-reduce) and adds the bias.

v2 design notes (vs v1 baseline):
  - All projection/score matmuls in bf16 (was fp32r): enables PE fast
    weight load (FWL), halves input DMA, same 1 cycle/row rate.
  - Additive mask folded into the score PSUM accumulation via a
    (-100*I) stationary matmul streaming the bf16 mask (was a
    multiplicative DVE/gpsimd pass over every exp output). exp of a
    masked score (~ -100) underflows to 0 in fp16. Frees DVE+Pool and
    removes exp->mask->ctx cross-engine chain per tile.
  - exp over [128, 2heads, 512] two-bank PSUM groups (N=1024/instr,
    was 2x N=512): fewer ACT pipeline fills.
  - Softmax reciprocal on ACT as exp(-ln(x)) (both fns live in the
    natural_log_exp_and_others table set => single table load); DVE
    reciprocal measured 3.4us/instr on HW.
  - Phase interleaving: output projection for q-chunk qc emitted right
    after attention of qc, overlapping the next chunk's attention.
  - PSUM budget: sc tag 2 banks x2 bufs + cy/cx 1 bank x2 bufs each
    = 8 banks. Phase-1 q/k use cy/cx slots, v uses sc slots.

Per-core PE moving-row budget ~400k rows (~170us at 1 row/cycle);
ACT exp 131072 elem/lane (~110us floor, ~150us with overheads).
"""

import math
from contextlib import ExitStack

import numpy as np
import ml_dtypes

import concourse.mybir as mybir
import concourse.tile as tile
from concourse import bacc
from concourse.bass_utils import run_bass_kernel_spmd

B, S, D, H = 2, 2048, 1024, 16
DH = D // H                 # 64
NCORES = 8
GROUPS = NCORES // B        # 4 head-groups per batch element
FPC = D // GROUPS           # 256 features (4 heads) per core
P = 128
SC = 512                    # q/s chunk (free dim of most matmuls)
NQC = S // SC               # 4
NKT = S // P                # 16 k-position tiles
NDT = D // P                # 8 contraction tiles over D

F32 = mybir.dt.float32
F16 = mybir.dt.float16
BF16 = mybir.dt.bfloat16

EXP = mybir.ActivationFunctionType.Exp
LN = mybir.ActivationFunctionType.Ln
MULT = mybir.AluOpType.mult

NEGMASK = -100.0


def _emit(ctx: ExitStack, tc: tile.TileContext, xT, wqT, wkT, wvT, woT,
          maskT, negI, outT):
    nc = tc.nc

    const = ctx.enter_context(tc.tile_pool(name="const", bufs=1))
    sb = ctx.enter_context(tc.tile_pool(name="sb", bufs=1))
    xtp = ctx.enter_context(tc.tile_pool(name="xtp", bufs=2))
    mkp = ctx.enter_context(tc.tile_pool(name="mkp", bufs=2))
    wp = ctx.enter_context(tc.tile_pool(name="wp", bufs=3))
    stg = ctx.enter_context(tc.tile_pool(name="stg", bufs=2))
    ps = ctx.enter_context(tc.tile_pool(name="ps", bufs=1, space="PSUM"))

    # ---- constants / weights in SBUF ----
    wq_sb = const.tile([P, NDT, FPC], BF16)
    nc.sync.dma_start(wq_sb[:], wqT[:])
    wk_sb = const.tile([P, NDT, FPC], BF16)
    nc.sync.dma_start(wk_sb[:], wkT[:])
    negI_sb = const.tile([P, P], BF16)
    nc.gpsimd.dma_start(negI_sb[:], negI[:])
    wv_sb = const.tile([P, NDT, FPC], BF16)
    nc.sync.dma_start(wv_sb[:], wvT[:])
    wo_sb = const.tile([P, FPC // P, D], BF16)
    nc.gpsimd.dma_start(wo_sb[:], woT[:])
    ones_bc = const.tile([P, DH], BF16)
    nc.vector.memset(ones_bc[:], 1.0 / DH)

    # ---- persistent activations ----
    q_sb = [sb.tile([P, S], BF16, name=f"q_sb{i}") for i in range(2)]
    k_sb = [sb.tile([P, S], BF16, name=f"k_sb{i}") for i in range(2)]
    v_sb = [sb.tile([P, 2, 192], F16, name=f"v_sb{i}") for i in range(NKT)]
    ctx_sb = [sb.tile([P, S], BF16, name=f"ctx_sb{i}") for i in range(2)]

    # ---- phase 1: projections ----
    for sc in range(NQC):
        xt = xtp.tile([P, NDT, SC], BF16, tag="xt", name=f"xt_{sc}")
        nc.sync.dma_start(xt[:], xT[sc])
        scl = slice(sc * SC, (sc + 1) * SC)
        for pair in range(2):
            fsl = slice(pair * P, (pair + 1) * P)
            qm = ps.tile([P, SC], F32, tag="cy", bufs=2, name=f"qm_{sc}_{pair}")
            km = ps.tile([P, SC], F32, tag="cx", bufs=2, name=f"km_{sc}_{pair}")
            for dt in range(NDT):
                nc.tensor.matmul(qm[:], wq_sb[:, dt, fsl], xt[:, dt, :],
                                 start=(dt == 0), stop=(dt == NDT - 1))
            for dt in range(NDT):
                nc.tensor.matmul(km[:], wk_sb[:, dt, fsl], xt[:, dt, :],
                                 start=(dt == 0), stop=(dt == NDT - 1))
            nc.vector.tensor_copy(q_sb[pair][:, scl], qm[:])
            nc.vector.tensor_copy(k_sb[pair][:, scl], km[:])
        for vg in range(2):  # two kt tiles per v psum tile
            vm = ps.tile([P, 2, FPC], F32, tag="sc", bufs=2, name=f"vm_{sc}_{vg}")
            for j in range(2):
                ssub = vg * 2 + j
                for dt in range(NDT):
                    nc.tensor.matmul(
                        vm[:, j, :],
                        xt[:, dt, ssub * P:(ssub + 1) * P],
                        wv_sb[:, dt, :],
                        start=(dt == 0), stop=(dt == NDT - 1))
            for j in range(2):
                kt = sc * 4 + vg * 2 + j
                src0 = vm[:, j, :].rearrange("p (pr f) -> p pr f", pr=2)
                nc.vector.tensor_copy(v_sb[kt][:, :, 0:DH], src0[:, :, 0:DH])
                nc.vector.tensor_copy(v_sb[kt][:, :, 2 * DH:3 * DH],
                                      src0[:, :, DH:2 * DH])
                nc.vector.memset(v_sb[kt][:, :, DH:2 * DH], 1.0)

    # ---- phases 2+3: attention, interleaved with output projection ----
    for qc in range(NQC):
        msk = mkp.tile([P, NKT, SC], BF16, tag="mask", name=f"msk_{qc}")
        nc.scalar.dma_start(msk[:], maskT[qc])
        qsl = slice(qc * SC, (qc + 1) * SC)
        for pair in range(2):
            cy = ps.tile([P, SC], F32, tag="cy", bufs=2, name=f"cy_{qc}_{pair}")
            cx = ps.tile([P, SC], F32, tag="cx", bufs=2, name=f"cx_{qc}_{pair}")
            for kt in range(NKT):
                ksl = slice(kt * P, (kt + 1) * P)
                sct = ps.tile([P, 2, SC], F32, tag="sc", bufs=2,
                              name=f"sct_{qc}_{pair}_{kt}")
                nc.tensor.matmul(sct[:, 0, :], k_sb[pair][0:DH, ksl],
                                 q_sb[pair][0:DH, qsl], start=True, stop=False)
                nc.tensor.matmul(sct[:, 1, :], k_sb[pair][DH:P, ksl],
                                 q_sb[pair][DH:P, qsl], start=True, stop=False,
                                 tile_position=(64, 0))
                nc.tensor.matmul(sct[:, 0, :], negI_sb[:], msk[:, kt, :],
                                 start=False, stop=True)
                nc.tensor.matmul(sct[:, 1, :], negI_sb[:], msk[:, kt, :],
                                 start=False, stop=True)
                w = wp.tile([P, 2, SC], F16, tag="w", name=f"w_{qc}_{pair}_{kt}")
                nc.scalar.activation(w[:], sct[:], EXP)
                vt = v_sb[kt]
                first, last = kt == 0, kt == NKT - 1
                nc.tensor.matmul(cy[:], vt[:, pair, 0:2 * DH], w[:, 0, :],
                                 start=first, stop=last)
                nc.tensor.matmul(cx[:], vt[:, pair, DH:3 * DH], w[:, 1, :],
                                 start=first, stop=last)
            # normalization: recip = exp(-ln(denom)) on ACT, partition
            # broadcast via ones matmul, multiply on DVE.
            lnt = stg.tile([P, SC], F32, tag="ln", name=f"ln_{qc}_{pair}")
            nc.scalar.activation(lnt[0:DH, :], cx[0:DH, :], LN)
            nc.scalar.activation(lnt[DH:P, :], cy[DH:P, :], LN)
            rcp = stg.tile([P, SC], BF16, tag="rcp", name=f"rcp_{qc}_{pair}")
            nc.scalar.activation(rcp[0:DH, :], lnt[0:DH, :], EXP, scale=-1.0)
            nc.scalar.activation(rcp[DH:P, :], lnt[DH:P, :], EXP, scale=-1.0)
            bc = ps.tile([P, SC], F32, tag="sc", bufs=2, name=f"bc_{qc}_{pair}")
            nc.tensor.matmul(bc[0:DH, :], ones_bc[DH:P, 0:DH], rcp[DH:P, :],
                             start=True, stop=True, tile_position=(64, 0))
            nc.tensor.matmul(bc[DH:P, :], ones_bc[0:DH, 0:DH], rcp[0:DH, :],
                             start=True, stop=True, tile_position=(0, 64))
            rcp2 = stg.tile([P, SC], F32, tag="rcp2", name=f"rcp2_{qc}_{pair}")
            nc.vector.tensor_copy(rcp2[0:DH, :], bc[0:DH, :])
            nc.vector.tensor_copy(rcp2[DH:P, :], bc[DH:P, :])
            nc.vector.tensor_tensor(ctx_sb[pair][0:DH, qsl], cy[0:DH, :],
                                    rcp2[0:DH, :], MULT)
            nc.vector.tensor_tensor(ctx_sb[pair][DH:P, qsl], cx[DH:P, :],
                                    rcp2[DH:P, :], MULT)

        # ---- phase 4 for this q-chunk ----
        for ft in range(D // P):
            om = ps.tile([P, SC], F32, tag=("cy", "cx")[ft % 2], bufs=2,
                         name=f"om_{qc}_{ft}")
            for ph in range(FPC // P):
                nc.tensor.matmul(om[:], wo_sb[:, ph, ft * P:(ft + 1) * P],
                                 ctx_sb[ph][:, qsl],
                                 start=(ph == 0), stop=(ph == FPC // P - 1))
            st = stg.tile([P, SC], F32, tag="st", name=f"st_{qc}_{ft}")
            nc.vector.tensor_copy(st[:], om[:])
            nc.gpsimd.dma_start(outT[ft, :, qc, :], st[:])


def build():
    nc = bacc.Bacc("TRN2", target_bir_lowering=False, debug=False,
                   num_devices=NCORES)
    # all inputs pre-tiled on the host so every DMA line is contiguous
    xT = nc.dram_tensor("xT", [NQC, P, NDT, SC], BF16, kind="ExternalInput").ap()
    wqT = nc.dram_tensor("wqT", [P, NDT, FPC], BF16, kind="ExternalInput").ap()
    wkT = nc.dram_tensor("wkT", [P, NDT, FPC], BF16, kind="ExternalInput").ap()
    wvT = nc.dram_tensor("wvT", [P, NDT, FPC], BF16, kind="ExternalInput").ap()
    woT = nc.dram_tensor("woT", [P, FPC // P, D], BF16, kind="ExternalInput").ap()
    maskT = nc.dram_tensor("maskT", [NQC, P, NKT, SC], BF16,
                           kind="ExternalInput").ap()
    negI = nc.dram_tensor("negI", [P, P], BF16, kind="ExternalInput").ap()
    outT = nc.dram_tensor("outT", [D // P, P, NQC, SC], F32,
                          kind="ExternalOutput").ap()
    with tile.TileContext(nc) as tc, ExitStack() as ctx:
        _emit(ctx, tc, xT, wqT, wkT, wvT, woT, maskT, negI, outT)
    nc.compile()
    return nc


def make_in_maps(query, mask, Wq, Wk, Wv, Wo):
    scale = 1.0 / math.sqrt(DH)
    bf16 = ml_dtypes.bfloat16
    negI = np.ascontiguousarray((np.eye(P, dtype=np.float32) * NEGMASK)
                                .astype(bf16))
    in_maps = []
    for b in range(B):
        # xT tiled: [NQC, P, NDT, SC]; element (sc, p, dt, s) = x[sc*SC+s, dt*P+p]
        xt = query[b].astype(np.float32).T.reshape(NDT, P, NQC, SC)
        xT = np.ascontiguousarray(xt.transpose(2, 1, 0, 3).astype(bf16))
        # mask tiled: [NQC, P, NKT, SC]; element (qc, p, kt, q) =
        #   1.0 if position (kt*P+p) is masked for query (qc*SC+q)
        mk = mask[b].T.astype(np.float32).reshape(NKT, P, NQC, SC)
        maskT = np.ascontiguousarray(mk.transpose(2, 1, 0, 3).astype(bf16))
        for g in range(GROUPS):
            f0 = g * FPC

            def pack_w(wT):  # [D, FPC] -> [P, NDT, FPC]
                return np.ascontiguousarray(
                    wT.reshape(NDT, P, FPC).transpose(1, 0, 2).astype(bf16))

            in_maps.append({
                "xT": xT,
                "wqT": pack_w((Wq[f0:f0 + FPC, :] * scale).T.astype(np.float32)),
                "wkT": pack_w(Wk[f0:f0 + FPC, :].T.astype(np.float32)),
                "wvT": pack_w(Wv[f0:f0 + FPC, :].T.astype(np.float32)),
                "woT": np.ascontiguousarray(
                    Wo[:, f0:f0 + FPC].T.astype(np.float32)
                    .reshape(FPC // P, P, D).transpose(1, 0, 2).astype(bf16)),
                "maskT": maskT,
                "negI": negI,
            })
    return in_maps


_NC_CACHE = {}


def _get_nc():
    if "nc" not in _NC_CACHE:
        _NC_CACHE["nc"] = build()
    return _NC_CACHE["nc"]


def gather(results, bo):
    out = np.empty((B, S, D), dtype=np.float32)
    for b in range(B):
        acc = results[b * GROUPS]["outT"].astype(np.float32).copy()
        for g in range(1, GROUPS):
            acc += results[b * GROUPS + g]["outT"]
        out[b] = acc.reshape(D, S).T + bo.astype(np.float32)
    return out


def kernel(query, mask, Wq, Wk, Wv, Wo, bo, **kwargs):
    nc = _get_nc()
    in_maps = make_in_maps(np.asarray(query), np.asarray(mask), np.asarray(Wq),
                           np.asarray(Wk), np.asarray(Wv), np.asarray(Wo))
    res = run_bass_kernel_spmd(nc, in_maps, list(range(NCORES)))
    return gather(res.results, np.asarray(bo))


# revision 13
# speedup vs baseline: 2.3597x; 1.0619x over previous
"""Multi-headed attention TRN2 Bass kernel (v2).

Problem: B=2, S=2048, D=1024, H=16 heads (dh=64), fp32 in/out, bool mask.

Sharding (8 cores): data-parallel over B (2) x tensor-parallel over heads
(4 heads / 256 features per core). Each core computes its head-group's
q/k/v projections, masked softmax attention, and a partial output
projection (Wo columns for its heads). Host sums the 4 partials per batch
element (the TP all-reduce) and adds the bias.

v2 design notes (vs v1 baseline):
  - All projection/score matmuls in bf16 (was fp32r): enables PE fast
    weight load (FWL), halves input DMA, same 1 cycle/row rate.
  - Additive mask folded into the score PSUM accumulation via a
    (-100*I) stationary matmul streaming the bf16 mask (was a
    multiplicative DVE/gpsimd pass over every exp output). exp of a
    masked score (~ -100) underflows to 0 in fp16. Frees DVE+Pool and
    removes exp->mask->ctx cross-engine chain per tile.
  - exp over [128, 2heads, 512] two-bank PSUM groups (N=1024/instr,
    was 2x N=512): fewer ACT pipeline fills.
  - Softmax reciprocal on ACT as exp(-ln(x)) (both fns live in the
    natural_log_exp_and_others table set => single table load); DVE
    reciprocal measured 3.4us/instr on HW.
  - Phase interleaving: output projection for q-chunk qc emitted right
    after attention of qc, overlapping the next chunk's attention.
  - PSUM budget: sc tag 2 banks x2 bufs + cy/cx 1 bank x2 bufs each
    = 8 banks. Phase-1 q/k use cy/cx slots, v uses sc slots.

Per-core PE moving-row budget ~400k rows (~170us at 1 row/cycle);
ACT exp 131072 elem/lane (~110us floor, ~150us with overheads).
"""

import math
from contextlib import ExitStack

import numpy as np
import ml_dtypes

import concourse.mybir as mybir
import concourse.tile as tile
from concourse import bacc
from concourse.bass_utils import run_bass_kernel_spmd

B, S, D, H = 2, 2048, 1024, 16
DH = D // H                 # 64
NCORES = 8
GROUPS = NCORES // B        # 4 head-groups per batch element
FPC = D // GROUPS           # 256 features (4 heads) per core
P = 128
SC = 512                    # q/s chunk (free dim of most matmuls)
NQC = S // SC               # 4
NKT = S // P                # 16 k-position tiles
NDT = D // P                # 8 contraction tiles over D

F32 = mybir.dt.float32
F16 = mybir.dt.float16
BF16 = mybir.dt.bfloat16

EXP = mybir.ActivationFunctionType.Exp
MULT = mybir.AluOpType.mult
F32R = mybir.dt.float32r

NEGMASK = -100.0


def _r(ap):
    return ap.bitcast(F32R)


def _emit(ctx: ExitStack, tc: tile.TileContext, xT, wqT, wkT, wvT, woT,
          maskT, negI, outT):
    nc = tc.nc

    const = ctx.enter_context(tc.tile_pool(name="const", bufs=1))
    sb = ctx.enter_context(tc.tile_pool(name="sb", bufs=1))
    xtp = ctx.enter_context(tc.tile_pool(name="xtp", bufs=2))
    mkp = ctx.enter_context(tc.tile_pool(name="mkp", bufs=2))
    wp = ctx.enter_context(tc.tile_pool(name="wp", bufs=3))
    stg = ctx.enter_context(tc.tile_pool(name="stg", bufs=2))
    ps = ctx.enter_context(tc.tile_pool(name="ps", bufs=1, space="PSUM"))

    # ---- constants / weights in SBUF ----
    wq_sb = const.tile([P, NDT, FPC], BF16)
    nc.sync.dma_start(wq_sb[:], wqT[:])
    wk_sb = const.tile([P, NDT, FPC], BF16)
    nc.sync.dma_start(wk_sb[:], wkT[:])
    negI_sb = const.tile([P, P], BF16)
    nc.gpsimd.dma_start(negI_sb[:], negI[:])
    wv_sb = const.tile([P, NDT, FPC], BF16)
    nc.gpsimd.dma_start(wv_sb[:], wvT[:])
    wo_sb = const.tile([P, FPC // P, D], BF16)
    nc.gpsimd.dma_start(wo_sb[:], woT[:])
    ones_bc = const.tile([P, DH], BF16)
    nc.vector.memset(ones_bc[:], 1.0 / DH)

    # ---- persistent activations ----
    q_sb = [sb.tile([P, S], BF16, name=f"q_sb{i}") for i in range(2)]
    k_sb = [sb.tile([P, S], BF16, name=f"k_sb{i}") for i in range(2)]
    v_sb = [sb.tile([P, 2, 192], F16, name=f"v_sb{i}") for i in range(NKT)]
    ctx_sb = [sb.tile([P, S], BF16, name=f"ctx_sb{i}") for i in range(2)]

    # ---- phase 1: projections ----
    for sc in range(NQC):
        xt = xtp.tile([P, NDT, SC], BF16, tag="xt", name=f"xt_{sc}")
        nc.sync.dma_start(xt[:], xT[sc])
        scl = slice(sc * SC, (sc + 1) * SC)
        for pair in range(2):
            fsl = slice(pair * P, (pair + 1) * P)
            qm = ps.tile([P, SC], F32, tag="cy", bufs=2, name=f"qm_{sc}_{pair}")
            km = ps.tile([P, SC], F32, tag="cx", bufs=2, name=f"km_{sc}_{pair}")
            for dt in range(NDT):
                nc.tensor.matmul(qm[:], wq_sb[:, dt, fsl], xt[:, dt, :],
                                 start=(dt == 0), stop=(dt == NDT - 1))
            for dt in range(NDT):
                nc.tensor.matmul(km[:], wk_sb[:, dt, fsl], xt[:, dt, :],
                                 start=(dt == 0), stop=(dt == NDT - 1))
            nc.vector.tensor_copy(q_sb[pair][:, scl], qm[:])
            nc.vector.tensor_copy(k_sb[pair][:, scl], km[:])
        for vg in range(2):  # two kt tiles per v psum tile
            vm = ps.tile([P, 2, FPC], F32, tag=("cy", "cx")[vg], bufs=2,
                         name=f"vm_{sc}_{vg}")
            for j in range(2):
                ssub = vg * 2 + j
                for dt in range(NDT):
                    nc.tensor.matmul(
                        vm[:, j, :],
                        xt[:, dt, ssub * P:(ssub + 1) * P],
                        wv_sb[:, dt, :],
                        start=(dt == 0), stop=(dt == NDT - 1))
            for j in range(2):
                kt = sc * 4 + vg * 2 + j
                src0 = vm[:, j, :].rearrange("p (pr f) -> p pr f", pr=2)
                nc.vector.tensor_copy(v_sb[kt][:, :, 0:DH], src0[:, :, 0:DH])
                nc.vector.tensor_copy(v_sb[kt][:, :, 2 * DH:3 * DH],
                                      src0[:, :, DH:2 * DH])
                nc.vector.memset(v_sb[kt][:, :, DH:2 * DH], 1.0)

    # ---- phases 2+3: attention, interleaved with output projection ----
    for qc in range(NQC):
        msk = mkp.tile([P, NKT, SC], BF16, tag="mask", name=f"msk_{qc}")
        nc.scalar.dma_start(msk[:], maskT[qc])
        qsl = slice(qc * SC, (qc + 1) * SC)
        for pair in range(2):
            cy = ps.tile([P, SC], F32, tag="cy", bufs=2, name=f"cy_{qc}_{pair}")
            cx = ps.tile([P, SC], F32, tag="cx", bufs=2, name=f"cx_{qc}_{pair}")
            for kt in range(NKT):
                ksl = slice(kt * P, (kt + 1) * P)
                sct = ps.tile([P, 2, SC], F32, tag="sc", bufs=2,
                              name=f"sct_{qc}_{pair}_{kt}")
                nc.tensor.matmul(sct[:, 0, :], k_sb[pair][0:DH, ksl],
                                 q_sb[pair][0:DH, qsl], start=True, stop=False)
                nc.tensor.matmul(sct[:, 1, :], k_sb[pair][DH:P, ksl],
                                 q_sb[pair][DH:P, qsl], start=True, stop=False,
                                 tile_position=(64, 0))
                nc.tensor.matmul(sct[:, 0, :], negI_sb[:], msk[:, kt, :],
                                 start=False, stop=True)
                nc.tensor.matmul(sct[:, 1, :], negI_sb[:], msk[:, kt, :],
                                 start=False, stop=True)
                w = wp.tile([P, 2, SC], F16, tag="w", name=f"w_{qc}_{pair}_{kt}")
                nc.scalar.activation(w[:], sct[:], EXP)
                vt = v_sb[kt]
                first, last = kt == 0, kt == NKT - 1
                nc.tensor.matmul(cy[:], vt[:, pair, 0:2 * DH], w[:, 0, :],
                                 start=first, stop=last)
                nc.tensor.matmul(cx[:], vt[:, pair, DH:3 * DH], w[:, 1, :],
                                 start=first, stop=last)
            # normalization: reciprocal of the denominators on DVE
            # (approx_fast: 51 ULP, ~5x cheaper than the iterative
            # reciprocal; ACT recip would thrash activation table sets),
            # partition broadcast via ones matmul, multiply on DVE.
            # reciprocal_approx_fast's custom-DVE uop mishandles
            # base_partition=64, so run it full-width per bank (the
            # halves holding ctx values produce garbage that is never
            # read) and slice the valid half afterwards.
            rcp = stg.tile([P, 2, SC], F32, tag="rcp", name=f"rcp_{qc}_{pair}")
            nc.vector.reciprocal_approx_fast(rcp[:, 0, :], cx[:])
            nc.vector.reciprocal_approx_fast(rcp[:, 1, :], cy[:])
            rcb = stg.tile([P, SC], BF16, tag="rcb", name=f"rcb_{qc}_{pair}")
            nc.vector.tensor_copy(rcb[0:DH, :], rcp[0:DH, 0, :])
            nc.vector.tensor_copy(rcb[DH:P, :], rcp[DH:P, 1, :])
            bc = ps.tile([P, SC], F32, tag="sc", bufs=2, name=f"bc_{qc}_{pair}")
            nc.tensor.matmul(bc[0:DH, :], ones_bc[DH:P, 0:DH], rcb[DH:P, :],
                             start=True, stop=True, tile_position=(64, 0))
            nc.tensor.matmul(bc[DH:P, :], ones_bc[0:DH, 0:DH], rcb[0:DH, :],
                             start=True, stop=True, tile_position=(0, 64))
            rcp2 = stg.tile([P, SC], F32, tag="rcp2", name=f"rcp2_{qc}_{pair}")
            nc.vector.tensor_copy(rcp2[0:DH, :], bc[0:DH, :])
            nc.vector.tensor_copy(rcp2[DH:P, :], bc[DH:P, :])
            nc.vector.tensor_tensor(ctx_sb[pair][0:DH, qsl], cy[0:DH, :],
                                    rcp2[0:DH, :], MULT)
            nc.vector.tensor_tensor(ctx_sb[pair][DH:P, qsl], cx[DH:P, :],
                                    rcp2[DH:P, :], MULT)

        # ---- phase 4 for this q-chunk ----
        for ft in range(D // P):
            om = ps.tile([P, SC], F32, tag=("cy", "cx")[ft % 2], bufs=2,
                         name=f"om_{qc}_{ft}")
            for ph in range(FPC // P):
                nc.tensor.matmul(om[:], wo_sb[:, ph, ft * P:(ft + 1) * P],
                                 ctx_sb[ph][:, qsl],
                                 start=(ph == 0), stop=(ph == FPC // P - 1))
            st = stg.tile([P, SC], F32, tag="st", name=f"st_{qc}_{ft}")
            nc.vector.tensor_copy(st[:], om[:])
            nc.gpsimd.dma_start(outT[ft, :, qc, :], st[:])


def build():
    nc = bacc.Bacc("TRN2", target_bir_lowering=False, debug=False,
                   num_devices=NCORES)
    # all inputs pre-tiled on the host so every DMA line is contiguous
    xT = nc.dram_tensor("xT", [NQC, P, NDT, SC], BF16, kind="ExternalInput").ap()
    wqT = nc.dram_tensor("wqT", [P, NDT, FPC], BF16, kind="ExternalInput").ap()
    wkT = nc.dram_tensor("wkT", [P, NDT, FPC], BF16, kind="ExternalInput").ap()
    wvT = nc.dram_tensor("wvT", [P, NDT, FPC], BF16, kind="ExternalInput").ap()
    woT = nc.dram_tensor("woT", [P, FPC // P, D], BF16, kind="ExternalInput").ap()
    maskT = nc.dram_tensor("maskT", [NQC, P, NKT, SC], BF16,
                           kind="ExternalInput").ap()
    negI = nc.dram_tensor("negI", [P, P], BF16, kind="ExternalInput").ap()
    outT = nc.dram_tensor("outT", [D // P, P, NQC, SC], F32,
                          kind="ExternalOutput").ap()
    with tile.TileContext(nc) as tc, ExitStack() as ctx:
        _emit(ctx, tc, xT, wqT, wkT, wvT, woT, maskT, negI, outT)
    nc.compile()
    return nc


def make_in_maps(query, mask, Wq, Wk, Wv, Wo):
    scale = 1.0 / math.sqrt(DH)
    bf16 = ml_dtypes.bfloat16
    negI = np.ascontiguousarray((np.eye(P, dtype=np.float32) * NEGMASK)
                                .astype(bf16))
    in_maps = []
    for b in range(B):
        # xT tiled: [NQC, P, NDT, SC]; element (sc, p, dt, s) = x[sc*SC+s, dt*P+p]
        xt = query[b].astype(np.float32).T.reshape(NDT, P, NQC, SC)
        xT = np.ascontiguousarray(xt.transpose(2, 1, 0, 3).astype(bf16))
        # mask tiled: [NQC, P, NKT, SC]; element (qc, p, kt, q) =
        #   1.0 if position (kt*P+p) is masked for query (qc*SC+q)
        mk = mask[b].T.astype(np.float32).reshape(NKT, P, NQC, SC)
        maskT = np.ascontiguousarray(mk.transpose(2, 1, 0, 3).astype(bf16))
        for g in range(GROUPS):
            f0 = g * FPC

            def pack_w(wT):  # [D, FPC] -> [P, NDT, FPC]
                return np.ascontiguousarray(
                    wT.reshape(NDT, P, FPC).transpose(1, 0, 2).astype(bf16))

            in_maps.append({
                "xT": xT,
                "wqT": pack_w((Wq[f0:f0 + FPC, :] * scale).T.astype(np.float32)),
                "wkT": pack_w(Wk[f0:f0 + FPC, :].T.astype(np.float32)),
                "wvT": pack_w(Wv[f0:f0 + FPC, :].T.astype(np.float32)),
                "woT": np.ascontiguousarray(
                    Wo[:, f0:f0 + FPC].T.astype(np.float32)
                    .reshape(FPC // P, P, D).transpose(1, 0, 2).astype(bf16)),
                "maskT": maskT,
                "negI": negI,
            })
    return in_maps


_NC_CACHE = {}


def _get_nc():
    if "nc" not in _NC_CACHE:
        _NC_CACHE["nc"] = build()
    return _NC_CACHE["nc"]


def gather(results, bo):
    out = np.empty((B, S, D), dtype=np.float32)
    for b in range(B):
        acc = results[b * GROUPS]["outT"].astype(np.float32).copy()
        for g in range(1, GROUPS):
            acc += results[b * GROUPS + g]["outT"]
        out[b] = acc.reshape(D, S).T + bo.astype(np.float32)
    return out


def kernel(query, mask, Wq, Wk, Wv, Wo, bo, **kwargs):
    nc = _get_nc()
    in_maps = make_in_maps(np.asarray(query), np.asarray(mask), np.asarray(Wq),
                           np.asarray(Wk), np.asarray(Wv), np.asarray(Wo))
    res = run_bass_kernel_spmd(nc, in_maps, list(range(NCORES)))
    return gather(res.results, np.asarray(bo))


# revision 17
# speedup vs baseline: 2.6176x; 1.1093x over previous
"""Multi-headed attention TRN2 Bass kernel (v2).

Problem: B=2, S=2048, D=1024, H=16 heads (dh=64), fp32 in/out, bool mask.

Sharding (8 cores): data-parallel over B (2) x tensor-parallel over heads
(4 heads / 256 features per core). Each core computes its head-group's
q/k/v projections, masked softmax attention, and a partial output
projection (Wo columns for its heads). Host sums the 4 partials per batch
element (the TP all-reduce) and adds the bias.

v2 design notes (vs v1 baseline):
  - All projection/score matmuls in bf16 (was fp32r): enables PE fast
    weight load (FWL), halves input DMA, same 1 cycle/row rate.
  - Additive mask folded into the score PSUM accumulation via a
    (-100*I) stationary matmul streaming the bf16 mask (was a
    multiplicative DVE/gpsimd pass over every exp output). exp of a
    masked score (~ -100) underflows to 0 in fp16. Frees DVE+Pool and
    removes exp->mask->ctx cross-engine chain per tile.
  - exp over [128, 2heads, 512] two-bank PSUM groups (N=1024/instr,
    was 2x N=512): fewer ACT pipeline fills.
  - Softmax reciprocal on ACT as exp(-ln(x)) (both fns live in the
    natural_log_exp_and_others table set => single table load); DVE
    reciprocal measured 3.4us/instr on HW.
  - Phase interleaving: output projection for q-chunk qc emitted right
    after attention of qc, overlapping the next chunk's attention.
  - PSUM budget: sc tag 2 banks x2 bufs + cy/cx 1 bank x2 bufs each
    = 8 banks. Phase-1 q/k use cy/cx slots, v uses sc slots.

Per-core PE moving-row budget ~400k rows (~170us at 1 row/cycle);
ACT exp 131072 elem/lane (~110us floor, ~150us with overheads).
"""

import math
from contextlib import ExitStack

import numpy as np
import ml_dtypes

import concourse.mybir as mybir
import concourse.tile as tile
from concourse import bacc
from concourse.bass_utils import run_bass_kernel_spmd

B, S, D, H = 2, 2048, 1024, 16
DH = D // H                 # 64
NCORES = 8
GROUPS = NCORES // B        # 4 head-groups per batch element
FPC = D // GROUPS           # 256 features (4 heads) per core
P = 128
SC = 512                    # q/s chunk (free dim of most matmuls)
NQC = S // SC               # 4
NKT = S // P                # 16 k-position tiles
NDT = D // P                # 8 contraction tiles over D

F32 = mybir.dt.float32
F16 = mybir.dt.float16
BF16 = mybir.dt.bfloat16

EXP = mybir.ActivationFunctionType.Exp
MULT = mybir.AluOpType.mult
F32R = mybir.dt.float32r

NEGMASK = -100.0


def _r(ap):
    return ap.bitcast(F32R)


def _emit(ctx: ExitStack, tc: tile.TileContext, xT, wqT, wkT, wvT, woT,
          maskT, negI, outT):
    nc = tc.nc

    const = ctx.enter_context(tc.tile_pool(name="const", bufs=1))
    sb = ctx.enter_context(tc.tile_pool(name="sb", bufs=1))
    xtp = ctx.enter_context(tc.tile_pool(name="xtp", bufs=2))
    mkp = ctx.enter_context(tc.tile_pool(name="mkp", bufs=2))
    wp = ctx.enter_context(tc.tile_pool(name="wp", bufs=3))
    stg = ctx.enter_context(tc.tile_pool(name="stg", bufs=2))
    ps = ctx.enter_context(tc.tile_pool(name="ps", bufs=1, space="PSUM"))

    # ---- constants / weights in SBUF ----
    # Ring assignment spreads the startup DMA burst: sync carries wq + x
    # chunks, gpsimd carries wk/wv/wo/negI, scalar carries the masks.
    wq_sb = const.tile([P, NDT, FPC], BF16)
    nc.sync.dma_start(wq_sb[:], wqT[:])
    wk_sb = const.tile([P, NDT, FPC], BF16)
    nc.gpsimd.dma_start(wk_sb[:], wkT[:])
    negI_sb = const.tile([P, P], BF16)
    nc.gpsimd.dma_start(negI_sb[:], negI[:])
    wv_sb = const.tile([P, NDT, FPC], BF16)
    nc.gpsimd.dma_start(wv_sb[:], wvT[:])
    wo_sb = const.tile([P, FPC // P, D], BF16)
    nc.gpsimd.dma_start(wo_sb[:], woT[:])
    ones_bc = const.tile([P, DH], BF16)
    nc.vector.memset(ones_bc[:], 1.0 / DH)

    # ---- persistent activations ----
    q_sb = [sb.tile([P, S], BF16, name=f"q_sb{i}") for i in range(2)]
    k_sb = [sb.tile([P, S], BF16, name=f"k_sb{i}") for i in range(2)]
    v_sb = [sb.tile([P, 2, 192], F16, name=f"v_sb{i}") for i in range(NKT)]
    ctx_sb = [sb.tile([P, S], BF16, name=f"ctx_sb{i}") for i in range(2)]

    msk_tiles = {}

    def get_msk(qc):
        if qc not in msk_tiles:
            t = mkp.tile([P, NKT, SC], BF16, tag="mask", name=f"msk_{qc}")
            nc.scalar.dma_start(t[:], maskT[qc])
            msk_tiles[qc] = t
        return msk_tiles[qc]

    get_msk(0)  # prefetch first mask during phase 1

    # ---- phase 1: projections ----
    for sc in range(NQC):
        xt = xtp.tile([P, NDT, SC], BF16, tag="xt", name=f"xt_{sc}")
        if sc == 0:
            # split the first chunk across two rings so the first
            # projection matmuls can start sooner
            nc.sync.dma_start(xt[:, 0:NDT // 2, :], xT[sc][:, 0:NDT // 2, :])
            nc.scalar.dma_start(xt[:, NDT // 2:NDT, :],
                                xT[sc][:, NDT // 2:NDT, :])
        else:
            nc.sync.dma_start(xt[:], xT[sc])
        scl = slice(sc * SC, (sc + 1) * SC)
        for pair in range(2):
            fsl = slice(pair * P, (pair + 1) * P)
            qm = ps.tile([P, SC], F32, tag="cy", bufs=2, name=f"qm_{sc}_{pair}")
            km = ps.tile([P, SC], F32, tag="cx", bufs=2, name=f"km_{sc}_{pair}")
            for dt in range(NDT):
                nc.tensor.matmul(qm[:], wq_sb[:, dt, fsl], xt[:, dt, :],
                                 start=(dt == 0), stop=(dt == NDT - 1))
            for dt in range(NDT):
                nc.tensor.matmul(km[:], wk_sb[:, dt, fsl], xt[:, dt, :],
                                 start=(dt == 0), stop=(dt == NDT - 1))
            nc.vector.tensor_copy(q_sb[pair][:, scl], qm[:])
            nc.vector.tensor_copy(k_sb[pair][:, scl], km[:])
        for vg in range(2):  # two kt tiles per v psum tile
            vm = ps.tile([P, 2, FPC], F32, tag=("cy", "cx")[vg], bufs=2,
                         name=f"vm_{sc}_{vg}")
            for j in range(2):
                ssub = vg * 2 + j
                for dt in range(NDT):
                    nc.tensor.matmul(
                        vm[:, j, :],
                        xt[:, dt, ssub * P:(ssub + 1) * P],
                        wv_sb[:, dt, :],
                        start=(dt == 0), stop=(dt == NDT - 1))
            for j in range(2):
                kt = sc * 4 + vg * 2 + j
                src0 = vm[:, j, :].rearrange("p (pr f) -> p pr f", pr=2)
                nc.vector.tensor_copy(v_sb[kt][:, :, 0:DH], src0[:, :, 0:DH])
                nc.vector.tensor_copy(v_sb[kt][:, :, 2 * DH:3 * DH],
                                      src0[:, :, DH:2 * DH])
                nc.vector.memset(v_sb[kt][:, :, DH:2 * DH], 1.0)

    # ---- phase 4 (per q-chunk helper) ----
    def phase4(qc):
        qsl = slice(qc * SC, (qc + 1) * SC)
        for ft in range(D // P):
            om = ps.tile([P, SC], F32, tag=("cy", "cx")[ft % 2], bufs=2,
                         name=f"om_{qc}_{ft}")
            for ph in range(FPC // P):
                nc.tensor.matmul(om[:], wo_sb[:, ph, ft * P:(ft + 1) * P],
                                 ctx_sb[ph][:, qsl],
                                 start=(ph == 0), stop=(ph == FPC // P - 1))
            st = stg.tile([P, SC], BF16, tag="st", name=f"st_{qc}_{ft}")
            nc.vector.tensor_copy(st[:], om[:])
            nc.gpsimd.dma_start(outT[ft, :, qc, :], st[:])

    # ---- phases 2+3: attention; phase 4 of chunk qc-1 is emitted after
    # pair 0 of chunk qc so its PSUM slots rotate without stalling either
    # the attention accumulators or the scores lookahead ----
    for qc in range(NQC):
        msk = get_msk(qc)
        qsl = slice(qc * SC, (qc + 1) * SC)
        for pair in range(2):
            cy = ps.tile([P, SC], F32, tag="cy", bufs=2, name=f"cy_{qc}_{pair}")
            cx = ps.tile([P, SC], F32, tag="cx", bufs=2, name=f"cx_{qc}_{pair}")
            for kt in range(NKT):
                ksl = slice(kt * P, (kt + 1) * P)
                sct = ps.tile([P, 2, SC], F32, tag="sc", bufs=2,
                              name=f"sct_{qc}_{pair}_{kt}")
                nc.tensor.matmul(sct[:, 0, :], k_sb[pair][0:DH, ksl],
                                 q_sb[pair][0:DH, qsl], start=True, stop=False)
                nc.tensor.matmul(sct[:, 1, :], k_sb[pair][DH:P, ksl],
                                 q_sb[pair][DH:P, qsl], start=True, stop=False,
                                 tile_position=(64, 0))
                nc.tensor.matmul(sct[:, 0, :], negI_sb[:], msk[:, kt, :],
                                 start=False, stop=True)
                nc.tensor.matmul(sct[:, 1, :], negI_sb[:], msk[:, kt, :],
                                 start=False, stop=True)
                w = wp.tile([P, 2, SC], F16, tag="w", name=f"w_{qc}_{pair}_{kt}")
                nc.scalar.activation(w[:], sct[:], EXP)
                vt = v_sb[kt]
                first, last = kt == 0, kt == NKT - 1
                nc.tensor.matmul(cy[:], vt[:, pair, 0:2 * DH], w[:, 0, :],
                                 start=first, stop=last)
                nc.tensor.matmul(cx[:], vt[:, pair, DH:3 * DH], w[:, 1, :],
                                 start=first, stop=last)
            # normalization: reciprocal of the denominators on DVE
            # (approx_fast: 51 ULP, ~5x cheaper than the iterative
            # reciprocal; ACT recip would thrash activation table sets),
            # partition broadcast via ones matmul, multiply on DVE.
            # reciprocal_approx_fast's custom-DVE uop mishandles
            # base_partition=64, so run it full-width per bank (the
            # halves holding ctx values produce garbage that is never
            # read) and slice the valid half afterwards.
            rcp = stg.tile([P, 2, SC], F32, tag="rcp", name=f"rcp_{qc}_{pair}")
            nc.vector.reciprocal_approx_fast(rcp[:, 0, :], cx[:])
            nc.vector.reciprocal_approx_fast(rcp[:, 1, :], cy[:])
            rcb = stg.tile([P, SC], BF16, tag="rcb", name=f"rcb_{qc}_{pair}")
            nc.vector.tensor_copy(rcb[0:DH, :], rcp[0:DH, 0, :])
            nc.vector.tensor_copy(rcb[DH:P, :], rcp[DH:P, 1, :])
            bc = ps.tile([P, SC], F32, tag="sc", bufs=2, name=f"bc_{qc}_{pair}")
            nc.tensor.matmul(bc[0:DH, :], ones_bc[DH:P, 0:DH], rcb[DH:P, :],
                             start=True, stop=True, tile_position=(64, 0))
            nc.tensor.matmul(bc[DH:P, :], ones_bc[0:DH, 0:DH], rcb[0:DH, :],
                             start=True, stop=True, tile_position=(0, 64))
            rcp2 = stg.tile([P, SC], F32, tag="rcp2", name=f"rcp2_{qc}_{pair}")
            nc.vector.tensor_copy(rcp2[0:DH, :], bc[0:DH, :])
            nc.vector.tensor_copy(rcp2[DH:P, :], bc[DH:P, :])
            nc.vector.tensor_tensor(ctx_sb[pair][0:DH, qsl], cy[0:DH, :],
                                    rcp2[0:DH, :], MULT)
            nc.vector.tensor_tensor(ctx_sb[pair][DH:P, qsl], cx[DH:P, :],
                                    rcp2[DH:P, :], MULT)
            if pair == 0:
                if qc > 0:
                    phase4(qc - 1)
                if qc + 1 < NQC:
                    get_msk(qc + 1)
    phase4(NQC - 1)


def build():
    nc = bacc.Bacc("TRN2", target_bir_lowering=False, debug=False,
                   num_devices=NCORES)
    # all inputs pre-tiled on the host so every DMA line is contiguous
    xT = nc.dram_tensor("xT", [NQC, P, NDT, SC], BF16, kind="ExternalInput").ap()
    wqT = nc.dram_tensor("wqT", [P, NDT, FPC], BF16, kind="ExternalInput").ap()
    wkT = nc.dram_tensor("wkT", [P, NDT, FPC], BF16, kind="ExternalInput").ap()
    wvT = nc.dram_tensor("wvT", [P, NDT, FPC], BF16, kind="ExternalInput").ap()
    woT = nc.dram_tensor("woT", [P, FPC // P, D], BF16, kind="ExternalInput").ap()
    maskT = nc.dram_tensor("maskT", [NQC, P, NKT, SC], BF16,
                           kind="ExternalInput").ap()
    negI = nc.dram_tensor("negI", [P, P], BF16, kind="ExternalInput").ap()
    outT = nc.dram_tensor("outT", [D // P, P, NQC, SC], BF16,
                          kind="ExternalOutput").ap()
    with tile.TileContext(nc) as tc, ExitStack() as ctx:
        _emit(ctx, tc, xT, wqT, wkT, wvT, woT, maskT, negI, outT)
    nc.compile()
    return nc


def make_in_maps(query, mask, Wq, Wk, Wv, Wo):
    scale = 1.0 / math.sqrt(DH)
    bf16 = ml_dtypes.bfloat16
    negI = np.ascontiguousarray((np.eye(P, dtype=np.float32) * NEGMASK)
                                .astype(bf16))
    in_maps = []
    for b in range(B):
        # xT tiled: [NQC, P, NDT, SC]; element (sc, p, dt, s) = x[sc*SC+s, dt*P+p]
        xt = query[b].astype(np.float32).T.reshape(NDT, P, NQC, SC)
        xT = np.ascontiguousarray(xt.transpose(2, 1, 0, 3).astype(bf16))
        # mask tiled: [NQC, P, NKT, SC]; element (qc, p, kt, q) =
        #   1.0 if position (kt*P+p) is masked for query (qc*SC+q)
        mk = mask[b].T.astype(np.float32).reshape(NKT, P, NQC, SC)
        maskT = np.ascontiguousarray(mk.transpose(2, 1, 0, 3).astype(bf16))
        for g in range(GROUPS):
            f0 = g * FPC

            def pack_w(wT):  # [D, FPC] -> [P, NDT, FPC]
                return np.ascontiguousarray(
                    wT.reshape(NDT, P, FPC).transpose(1, 0, 2).astype(bf16))

            in_maps.append({
                "xT": xT,
                "wqT": pack_w((Wq[f0:f0 + FPC, :] * scale).T.astype(np.float32)),
                "wkT": pack_w(Wk[f0:f0 + FPC, :].T.astype(np.float32)),
                "wvT": pack_w(Wv[f0:f0 + FPC, :].T.astype(np.float32)),
                "woT": np.ascontiguousarray(
                    Wo[:, f0:f0 + FPC].T.astype(np.float32)
                    .reshape(FPC // P, P, D).transpose(1, 0, 2).astype(bf16)),
                "maskT": maskT,
                "negI": negI,
            })
    return in_maps


_NC_CACHE = {}


def _get_nc():
    if "nc" not in _NC_CACHE:
        _NC_CACHE["nc"] = build()
    return _NC_CACHE["nc"]


def gather(results, bo):
    out = np.empty((B, S, D), dtype=np.float32)
    for b in range(B):
        acc = results[b * GROUPS]["outT"].astype(np.float32).copy()
        for g in range(1, GROUPS):
            acc += results[b * GROUPS + g]["outT"]
        out[b] = acc.reshape(D, S).T + bo.astype(np.float32)
    return out


def kernel(query, mask, Wq, Wk, Wv, Wo, bo, **kwargs):
    nc = _get_nc()
    in_maps = make_in_maps(np.asarray(query), np.asarray(mask), np.asarray(Wq),
                           np.asarray(Wk), np.asarray(Wv), np.asarray(Wo))
    res = run_bass_kernel_spmd(nc, in_maps, list(range(NCORES)))
    return gather(res.results, np.asarray(bo))
